# revision 30
# baseline (speedup 1.0000x reference)
"""Multi-head attention (b=4, l=2048, d=1024, 16 heads) on 8 TRN2 NeuronCores.

Sharding: data parallel over the 4 batches x tensor parallel over 2 head
groups (8 heads each). core = 2*batch + head_group. Each core computes its
batch's attention for its 8 heads plus the partial W_o projection
(row-parallel); the host sums the two partials per batch and adds b_o.
No on-chip collectives needed.

Per-core layout (everything transposed so features sit on partitions):
  xT  [1024, 2048]; Q^T,K^T [512, 2048]; V in [m, o] layout.
  Per head pair (sharing an o-partition tile, rows 0-63 / 64-127):
    S^T = K Q^T for both heads into one [128, 1024] PSUM tile
    -> one exp on ScalarE -> P^T bf16
    -> A^T accumulated via V_aug (64 V cols + ones col -> denom in row 64)
  softmax denominators are DMA-scattered onto separate partitions and
  reciprocal'd in one batched DVE op; broadcast back via gpsimd.
  Y_partial = A^T.T-contract @ WoT -> [2048, 1024] fp32.

All DRAM operands are host-pretiled so every DMA is contiguous per
partition (32KB runs instead of 4KB), and emission interleaves the next
pair's QK projection / the O projection into the ACT-bound attention
stream so the TensorEngine never idles the ScalarEngine at phase edges.
"""

import os

import numpy as np

B = 4
L = 2048
D = 1024
P = 128
NH = 8          # heads per core
DH = 64
O = NH * DH     # 512 qkv dims per core
DT = D // P     # 8 d-tiles
OT = O // P     # 4 o-tiles (= head pairs)
MT = L // P     # 16 m-tiles
LC4 = L // 512  # 4 l-chunks of 512

_cache = {}


def _build():
    import concourse.tile as tile
    from concourse import bacc, mybir

    nc = bacc.Bacc("TRN2", target_bir_lowering=False, debug=False)
    bf = mybir.dt.bfloat16
    f32 = mybir.dt.float32
    Exp = mybir.ActivationFunctionType.Exp

    # host-pretiled: row p holds the concatenation over tiles (see
    # make_in_maps) so per-partition DRAM runs are large and contiguous.
    xT = nc.dram_tensor("xT", [P, DT * L], bf, kind="ExternalInput").ap()
    wqT = nc.dram_tensor("wqT", [P, DT * O], bf, kind="ExternalInput").ap()
    wkT = nc.dram_tensor("wkT", [P, DT * O], bf, kind="ExternalInput").ap()
    wvT = nc.dram_tensor("wvT", [P, DT * O], bf, kind="ExternalInput").ap()
    woT = nc.dram_tensor("woT", [P, OT * D], bf, kind="ExternalInput").ap()
    out = nc.dram_tensor("out", [P, MT * D], f32, kind="ExternalOutput").ap()
    xT3 = xT.rearrange("p (dt l) -> p dt l", l=L)
    out3 = out.rearrange("p (lt j) -> p lt j", j=D)

    with tile.TileContext(nc) as tc:
        with (
            tc.tile_pool(name="persist", bufs=1) as pp,
            tc.tile_pool(name="work", bufs=4) as wp,
            tc.tile_pool(name="psum", bufs=1, space="PSUM") as psp,
        ):
            xT_sb = pp.tile([P, DT, L], bf, name="xT_sb")
            wq_sb = pp.tile([P, DT, O], bf, name="wq_sb")
            wk_sb = pp.tile([P, DT, O], bf, name="wk_sb")
            wv_sb = pp.tile([P, DT, O], bf, name="wv_sb")
            wo_sb = pp.tile([P, OT, D], bf, name="wo_sb")
            qT_sb = pp.tile([P, OT, L], bf, name="qT_sb")
            kT_sb = pp.tile([P, OT, L], bf, name="kT_sb")
            vaug_sb = pp.tile([P, MT, NH * 65], bf, name="vaug_sb")
            at_sb = pp.tile([P, OT, L], bf, name="at_sb")

            nc.sync.dma_start(wq_sb[:], wqT[:].rearrange("p (dt o) -> p dt o", o=O))
            nc.sync.dma_start(wk_sb[:], wkT[:].rearrange("p (dt o) -> p dt o", o=O))
            for i in range(4):
                nc.sync.dma_start(xT_sb[:, 2 * i:2 * i + 2, :], xT3[:, 2 * i:2 * i + 2, :])
            nc.sync.dma_start(wv_sb[:], wvT[:].rearrange("p (dt o) -> p dt o", o=O))
            nc.sync.dma_start(wo_sb[:], woT[:].rearrange("p (ot j) -> p ot j", j=D))

            # ones columns for the softmax-denominator trick; V copies below
            # overwrite the first 64 columns of each head's 65-column block.
            nc.gpsimd.memset(vaug_sb[:], 1.0)

            def proj_qk_group(w_sb, dst, ot, lc):
                ps = psp.tile([P, 512], f32, tag="mm512", bufs=2, name="ps_qk")
                for dt in range(DT):
                    nc.tensor.matmul(
                        ps[:],
                        w_sb[:, dt, ot * P:(ot + 1) * P],
                        xT_sb[:, dt, lc * 512:(lc + 1) * 512],
                        start=(dt == 0),
                        stop=(dt == DT - 1),
                    )
                nc.vector.tensor_copy(dst[:, ot, lc * 512:(lc + 1) * 512], ps[:])

            def proj_v(mt):
                ps = psp.tile([P, 512], f32, tag="mm512", bufs=2, name="ps_v")
                for dt in range(DT):
                    nc.tensor.matmul(
                        ps[:],
                        xT_sb[:, dt, mt * P:(mt + 1) * P],
                        wv_sb[:, dt, :],
                        start=(dt == 0),
                        stop=(dt == DT - 1),
                    )
                for h in range(NH):
                    nc.vector.tensor_copy(
                        vaug_sb[:, mt, h * 65:h * 65 + 64],
                        ps[:, h * DH:(h + 1) * DH],
                    )

            def proj_o(lt):
                ob = wp.tile([P, 1024], f32, tag="ob", bufs=3, name="ob")
                for jc in range(2):
                    ps = psp.tile([P, 512], f32, tag="mm512", bufs=2, name="ps_o")
                    for ot in range(OT):
                        nc.tensor.matmul(
                            ps[:],
                            at_sb[:, ot, lt * P:(lt + 1) * P],
                            wo_sb[:, ot, jc * 512:(jc + 1) * 512],
                            start=(ot == 0),
                            stop=(ot == OT - 1),
                        )
                    nc.vector.tensor_copy(ob[:, jc * 512:(jc + 1) * 512], ps[:])
                nc.sync.dma_start(out3[:, lt, :], ob[:])

            # Only the two groups the very first S^T matmul needs are
            # emitted upfront; everything else streams in as filler work
            # inside the attention mt loops.
            proj_qk_group(wq_sb, qT_sb, 0, 0)
            proj_qk_group(wk_sb, kT_sb, 0, 0)

            def normalize(pair, lc, rd, r, rb):
                """at_sb[rb:rb+64, pair, lc-chunk] *= broadcast(rd[r])."""
                cols = slice(lc * 512, (lc + 1) * 512)
                rstg = wp.tile([1, 512], f32, tag="rstg", bufs=4, name="rstg")
                nc.sync.dma_start(rstg[:], rd[r:r + 1, :])
                rbb = wp.tile([P, 512], f32, tag="rbb", bufs=4, name="rbb")
                nc.gpsimd.partition_broadcast(rbb[:], rstg[:])
                nc.vector.tensor_mul(
                    out=at_sb[rb:rb + DH, pair, cols],
                    in0=at_sb[rb:rb + DH, pair, cols],
                    in1=rbb[rb:rb + DH, :],
                )

            # ---- attention, one head pair at a time -----------------------
            # Filler emissions (projection groups / O-projection tiles) are
            # woven INSIDE the mt loops so the PE always has matmul work
            # during the ACT-bound exp stream, never in bursts that stall
            # the ScalarEngine at chunk boundaries.
            def fillers_for(pair, lc):
                f = {}
                if pair == 0:
                    # pair 0's own remaining groups: kT m-groups just in
                    # time (S at m-tile mt reads kT group mt//4), qT for
                    # the next l-chunk midway through the previous one.
                    if lc == 0:
                        for g in (1, 2, 3):
                            f.setdefault(4 * g - 1, []).append(
                                lambda g=g: proj_qk_group(wk_sb, kT_sb, 0, g))
                    if lc < 3:
                        f.setdefault(6, []).append(
                            lambda lc=lc: proj_qk_group(wq_sb, qT_sb, 0, lc + 1))
                if pair < OT - 1:
                    # next pair's 8 groups: two per l-chunk, k-groups first
                    nxt = pair + 1
                    order = [(wq_sb, qT_sb, 0), (wk_sb, kT_sb, 0),
                             (wk_sb, kT_sb, 1), (wk_sb, kT_sb, 2),
                             (wk_sb, kT_sb, 3), (wq_sb, qT_sb, 1),
                             (wq_sb, qT_sb, 2), (wq_sb, qT_sb, 3)]
                    for i in (2 * lc, 2 * lc + 1):
                        w_sb, dst, g = order[i]
                        f.setdefault(5 + 6 * (i % 2), []).append(
                            lambda w_sb=w_sb, dst=dst, g=g:
                                proj_qk_group(w_sb, dst, nxt, g))
                if pair == OT - 1 and lc > 0:
                    # O-projection tiles unlocked by the previous l-chunk's
                    # normalize, one every four m-tiles.
                    for i, lt in enumerate(range(4 * (lc - 1), 4 * lc)):
                        f.setdefault(4 * i + 3, []).append(
                            lambda lt=lt: proj_o(lt))
                return f

            for pair in range(OT):
                h0, h1 = 2 * pair, 2 * pair + 1
                last = pair == OT - 1
                if not last:
                    dn = wp.tile([8, 512], f32, tag="dn", bufs=2, name="dn")
                for lc in range(LC4):
                    cols = slice(lc * 512, (lc + 1) * 512)
                    fill = fillers_for(pair, lc)
                    av0 = psp.tile([P, 512], f32, tag="av", bufs=2, name="av0")
                    av1 = psp.tile([P, 512], f32, tag="av", bufs=2, name="av1")
                    for mt in range(MT):
                        mcols = slice(mt * P, (mt + 1) * P)
                        s = psp.tile([P, 1024], f32, tag="s", bufs=2, name="s")
                        nc.tensor.matmul(
                            s[:, 0:512],
                            kT_sb[0:DH, pair, mcols],
                            qT_sb[0:DH, pair, cols],
                            start=True, stop=True,
                        )
                        nc.tensor.matmul(
                            s[:, 512:1024],
                            kT_sb[DH:2 * DH, pair, mcols],
                            qT_sb[DH:2 * DH, pair, cols],
                            start=True, stop=True,
                        )
                        p = wp.tile([P, 1024], bf, tag="p", bufs=8, name="p")
                        nc.scalar.activation(p[:], s[:], Exp, scale=0.125)
                        if pair == 0 and lc == 0:
                            proj_v(mt)
                        nc.tensor.matmul(
                            av0[0:65, :],
                            vaug_sb[:, mt, h0 * 65:h0 * 65 + 65],
                            p[:, 0:512],
                            start=(mt == 0), stop=(mt == MT - 1),
                        )
                        nc.tensor.matmul(
                            av1[0:65, :],
                            vaug_sb[:, mt, h1 * 65:h1 * 65 + 65],
                            p[:, 512:1024],
                            start=(mt == 0), stop=(mt == MT - 1),
                        )
                        for fn in fill.get(mt, ()):
                            fn()
                    # stash unnormalized A^T; scatter denominator rows for
                    # a batched reciprocal (per-pair; the last pair uses a
                    # per-lc [2,512] tile so it can normalize immediately,
                    # and its final chunk takes the shortest path: direct
                    # reciprocal + multiply straight from PSUM).
                    tail = last and lc == LC4 - 1
                    if last and not tail:
                        dnl = wp.tile([2, 512], f32, tag="dnl", bufs=2, name="dnl")
                    for hidx, (rb, av) in enumerate(((0, av0), (DH, av1))):
                        if tail:
                            rt = wp.tile([1, 512], f32, tag="rt", bufs=2, name="rt")
                            nc.vector.reciprocal(rt[:], av[64:65, :])
                            rbb = wp.tile([DH, 512], f32, tag="rbbt", bufs=2, name="rbbt")
                            nc.gpsimd.partition_broadcast(rbb[:], rt[:])
                            nc.vector.tensor_mul(
                                out=at_sb[rb:rb + DH, pair, cols],
                                in0=av[0:DH, :],
                                in1=rbb[:],
                            )
                            continue
                        nc.vector.tensor_copy(
                            at_sb[rb:rb + DH, pair, cols], av[0:DH, :]
                        )
                        stg = wp.tile([1, 512], f32, tag="stg", bufs=4, name="stg")
                        nc.vector.tensor_copy(stg[:], av[64:65, :])
                        if last:
                            nc.sync.dma_start(dnl[hidx:hidx + 1, :], stg[:])
                        else:
                            nc.sync.dma_start(dn[2 * lc + hidx:2 * lc + hidx + 1, :], stg[:])
                    if last and not tail:
                        rdl = wp.tile([2, 512], f32, tag="rdl", bufs=2, name="rdl")
                        nc.vector.reciprocal(rdl[:], dnl[:])
                        for hidx, rb in ((0, 0), (1, DH)):
                            normalize(pair, lc, rdl, hidx, rb)
                if not last:
                    rd = wp.tile([8, 512], f32, tag="rd", bufs=2, name="rd")
                    nc.vector.reciprocal(rd[:], dn[:])
                    for lc in range(LC4):
                        for hidx, rb in ((0, 0), (1, DH)):
                            normalize(pair, lc, rd, 2 * lc + hidx, rb)
            for lt in range(12, 16):
                proj_o(lt)

    nc.compile()
    return nc


def get_nc():
    if "nc" not in _cache:
        _cache["nc"] = _build()
    return _cache["nc"]


def _pretile(a, p=P):
    """[T*p, F] -> [p, T*F] with row i holding concat over tiles t of a[t*p+i]."""
    t = a.shape[0] // p
    return np.ascontiguousarray(
        a.reshape(t, p, a.shape[1]).transpose(1, 0, 2).reshape(p, t * a.shape[1])
    )


def make_in_maps(x, W_q, W_k, W_v, W_o):
    import ml_dtypes

    bf = ml_dtypes.bfloat16
    x = np.asarray(x, dtype=np.float32)
    W_q = np.asarray(W_q, dtype=np.float32)
    W_k = np.asarray(W_k, dtype=np.float32)
    W_v = np.asarray(W_v, dtype=np.float32)
    W_o = np.asarray(W_o, dtype=np.float32)

    in_maps = []
    for core in range(8):
        b, hg = divmod(core, 2)
        rows = slice(hg * O, (hg + 1) * O)
        in_maps.append({
            "xT": _pretile(np.ascontiguousarray(x[b].T)).astype(bf),
            "wqT": _pretile(np.ascontiguousarray(W_q[rows].T)).astype(bf),
            "wkT": _pretile(np.ascontiguousarray(W_k[rows].T)).astype(bf),
            "wvT": _pretile(np.ascontiguousarray(W_v[rows].T)).astype(bf),
            "woT": _pretile(np.ascontiguousarray(W_o[:, rows].T)).astype(bf),
        })
    return in_maps


def kernel(x, W_q, W_k, W_v, W_o, b_o):
    from concourse.bass_utils import run_bass_kernel_spmd

    nc = get_nc()
    in_maps = make_in_maps(x, W_q, W_k, W_v, W_o)
    trace = bool(int(os.environ.get("ATTN_TRACE", "0")))
    res = run_bass_kernel_spmd(nc, in_maps, core_ids=list(range(8)), trace=trace)
    if trace and res.exec_time_ns is not None:
        _cache["exec_time_ns"] = res.exec_time_ns
        _cache["mean_exec_time_ns"] = res.mean_exec_time_ns

    b_o = np.asarray(b_o, dtype=np.float32)
    out = np.empty((B, L, D), np.float32)
    for b in range(B):
        # out dram is [128, 16, 1024]: row p, tile lt -> token lt*128+p
        acc = (res.results[2 * b]["out"] + res.results[2 * b + 1]["out"])
        out[b] = acc.reshape(P, MT, D).transpose(1, 0, 2).reshape(L, D) + b_o
    return out


# revision 31
# speedup vs baseline: 1.0122x; 1.0122x over previous
"""Multi-head attention (b=4, l=2048, d=1024, 16 heads) on 8 TRN2 NeuronCores.

Sharding: data parallel over the 4 batches x tensor parallel over 2 head
groups (8 heads each). core = 2*batch + head_group. Each core computes its
batch's attention for its 8 heads plus the partial W_o projection
(row-parallel); the host sums the two partials per batch and adds b_o.
No on-chip collectives needed.

Per-core layout (everything transposed so features sit on partitions):
  xT  [1024, 2048]; Q^T,K^T [512, 2048]; V in [m, o] layout.
  Per head pair (sharing an o-partition tile, rows 0-63 / 64-127):
    S^T = K Q^T for both heads into one [128, 1024] PSUM tile
    -> one exp on ScalarE -> P^T bf16
    -> A^T accumulated via V_aug (64 V cols + ones col -> denom in row 64)
  softmax denominators are DMA-scattered onto separate partitions and
  reciprocal'd in one batched DVE op; broadcast back via gpsimd.
  Y_partial = A^T.T-contract @ WoT -> [2048, 1024] fp32.

All DRAM operands are host-pretiled so every DMA is contiguous per
partition (32KB runs instead of 4KB), and emission interleaves the next
pair's QK projection / the O projection into the ACT-bound attention
stream so the TensorEngine never idles the ScalarEngine at phase edges.
"""

import os

import numpy as np

B = 4
L = 2048
D = 1024
P = 128
NH = 8          # heads per core
DH = 64
O = NH * DH     # 512 qkv dims per core
DT = D // P     # 8 d-tiles
OT = O // P     # 4 o-tiles (= head pairs)
MT = L // P     # 16 m-tiles
LC4 = L // 512  # 4 l-chunks of 512

_cache = {}


def _build():
    import concourse.tile as tile
    from concourse import bacc, mybir

    nc = bacc.Bacc("TRN2", target_bir_lowering=False, debug=False)
    bf = mybir.dt.bfloat16
    f32 = mybir.dt.float32
    Exp = mybir.ActivationFunctionType.Exp

    # host-pretiled: row p holds the concatenation over tiles (see
    # make_in_maps) so per-partition DRAM runs are large and contiguous.
    xT = nc.dram_tensor("xT", [P, DT * L], bf, kind="ExternalInput").ap()
    wqT = nc.dram_tensor("wqT", [P, DT * O], bf, kind="ExternalInput").ap()
    wkT = nc.dram_tensor("wkT", [P, DT * O], bf, kind="ExternalInput").ap()
    wvT = nc.dram_tensor("wvT", [P, DT * O], bf, kind="ExternalInput").ap()
    woT = nc.dram_tensor("woT", [P, OT * D], bf, kind="ExternalInput").ap()
    out = nc.dram_tensor("out", [P, MT * D], f32, kind="ExternalOutput").ap()
    xT3 = xT.rearrange("p (dt l) -> p dt l", l=L)
    out3 = out.rearrange("p (lt j) -> p lt j", j=D)

    with tile.TileContext(nc) as tc:
        with (
            tc.tile_pool(name="persist", bufs=1) as pp,
            tc.tile_pool(name="work", bufs=3) as wp,
            tc.tile_pool(name="psum", bufs=1, space="PSUM") as psp,
        ):
            xT_sb = pp.tile([P, DT, L], bf, name="xT_sb")
            wq_sb = pp.tile([P, DT, O], bf, name="wq_sb")
            wk_sb = pp.tile([P, DT, O], bf, name="wk_sb")
            wv_sb = pp.tile([P, DT, O], bf, name="wv_sb")
            wo_sb = pp.tile([P, OT, D], bf, name="wo_sb")
            qT_sb = pp.tile([P, OT, L], bf, name="qT_sb")
            kT_sb = pp.tile([P, OT, L], bf, name="kT_sb")
            vaug_sb = pp.tile([P, MT, NH * 65], bf, name="vaug_sb")
            at_sb = pp.tile([P, OT, L], bf, name="at_sb")

            nc.sync.dma_start(wq_sb[:], wqT[:].rearrange("p (dt o) -> p dt o", o=O))
            nc.sync.dma_start(wk_sb[:], wkT[:].rearrange("p (dt o) -> p dt o", o=O))
            for i in range(4):
                nc.sync.dma_start(xT_sb[:, 2 * i:2 * i + 2, :], xT3[:, 2 * i:2 * i + 2, :])
            nc.sync.dma_start(wv_sb[:], wvT[:].rearrange("p (dt o) -> p dt o", o=O))
            nc.sync.dma_start(wo_sb[:], woT[:].rearrange("p (ot j) -> p ot j", j=D))

            # ones columns for the softmax-denominator trick; V copies below
            # overwrite the first 64 columns of each head's 65-column block.
            nc.gpsimd.memset(vaug_sb[:], 1.0)

            def proj_qk_group(w_sb, dst, ot, lc):
                ps = psp.tile([P, 512], f32, tag="mm512", bufs=2, name="ps_qk")
                for dt in range(DT):
                    nc.tensor.matmul(
                        ps[:],
                        w_sb[:, dt, ot * P:(ot + 1) * P],
                        xT_sb[:, dt, lc * 512:(lc + 1) * 512],
                        start=(dt == 0),
                        stop=(dt == DT - 1),
                    )
                nc.vector.tensor_copy(dst[:, ot, lc * 512:(lc + 1) * 512], ps[:])

            def proj_v(mt):
                ps = psp.tile([P, 512], f32, tag="mm512", bufs=2, name="ps_v")
                for dt in range(DT):
                    nc.tensor.matmul(
                        ps[:],
                        xT_sb[:, dt, mt * P:(mt + 1) * P],
                        wv_sb[:, dt, :],
                        start=(dt == 0),
                        stop=(dt == DT - 1),
                    )
                for h in range(NH):
                    nc.vector.tensor_copy(
                        vaug_sb[:, mt, h * 65:h * 65 + 64],
                        ps[:, h * DH:(h + 1) * DH],
                    )

            def proj_o(lt):
                ob = wp.tile([P, 1024], f32, tag="ob", bufs=3, name="ob")
                for jc in range(2):
                    ps = psp.tile([P, 512], f32, tag="mm512", bufs=2, name="ps_o")
                    for ot in range(OT):
                        nc.tensor.matmul(
                            ps[:],
                            at_sb[:, ot, lt * P:(lt + 1) * P],
                            wo_sb[:, ot, jc * 512:(jc + 1) * 512],
                            start=(ot == 0),
                            stop=(ot == OT - 1),
                        )
                    nc.vector.tensor_copy(ob[:, jc * 512:(jc + 1) * 512], ps[:])
                nc.sync.dma_start(out3[:, lt, :], ob[:])

            # Only the two groups the very first S^T matmul needs are
            # emitted upfront; everything else streams in as filler work
            # inside the attention mt loops.
            proj_qk_group(wq_sb, qT_sb, 0, 0)
            proj_qk_group(wk_sb, kT_sb, 0, 0)

            def normalize(pair, lc, rd, r, rb):
                """at_sb[rb:rb+64, pair, lc-chunk] *= broadcast(rd[r])."""
                cols = slice(lc * 512, (lc + 1) * 512)
                rstg = wp.tile([1, 512], f32, tag="rstg", bufs=4, name="rstg")
                nc.sync.dma_start(rstg[:], rd[r:r + 1, :])
                rbb = wp.tile([P, 512], f32, tag="rbb", bufs=4, name="rbb")
                nc.gpsimd.partition_broadcast(rbb[:], rstg[:])
                nc.vector.tensor_mul(
                    out=at_sb[rb:rb + DH, pair, cols],
                    in0=at_sb[rb:rb + DH, pair, cols],
                    in1=rbb[rb:rb + DH, :],
                )

            # ---- attention, one head pair at a time -----------------------
            # Filler emissions (projection groups / O-projection tiles) are
            # woven INSIDE the mt loops so the PE always has matmul work
            # during the ACT-bound exp stream, never in bursts that stall
            # the ScalarEngine at chunk boundaries.
            def fillers_for(pair, lc):
                f = {}
                if pair == 0:
                    # pair 0's own remaining groups: kT m-groups just in
                    # time (S at m-tile mt reads kT group mt//4), qT for
                    # the next l-chunk midway through the previous one.
                    if lc == 0:
                        for g in (1, 2, 3):
                            f.setdefault(4 * g - 1, []).append(
                                lambda g=g: proj_qk_group(wk_sb, kT_sb, 0, g))
                    if lc < 3:
                        f.setdefault(6, []).append(
                            lambda lc=lc: proj_qk_group(wq_sb, qT_sb, 0, lc + 1))
                if pair < OT - 1:
                    # next pair's 8 groups: two per l-chunk, k-groups first
                    nxt = pair + 1
                    order = [(wq_sb, qT_sb, 0), (wk_sb, kT_sb, 0),
                             (wk_sb, kT_sb, 1), (wk_sb, kT_sb, 2),
                             (wk_sb, kT_sb, 3), (wq_sb, qT_sb, 1),
                             (wq_sb, qT_sb, 2), (wq_sb, qT_sb, 3)]
                    for i in (2 * lc, 2 * lc + 1):
                        w_sb, dst, g = order[i]
                        f.setdefault(5 + 6 * (i % 2), []).append(
                            lambda w_sb=w_sb, dst=dst, g=g:
                                proj_qk_group(w_sb, dst, nxt, g))
                if pair == OT - 1 and lc > 0:
                    # O-projection tiles unlocked by the previous l-chunk's
                    # normalize, one every four m-tiles.
                    for i, lt in enumerate(range(4 * (lc - 1), 4 * lc)):
                        f.setdefault(4 * i + 3, []).append(
                            lambda lt=lt: proj_o(lt))
                return f

            for pair in range(OT):
                h0, h1 = 2 * pair, 2 * pair + 1
                last = pair == OT - 1
                if not last:
                    dn = wp.tile([8, 512], f32, tag="dn", bufs=2, name="dn")
                for lc in range(LC4):
                    cols = slice(lc * 512, (lc + 1) * 512)
                    fill = fillers_for(pair, lc)
                    av0 = psp.tile([P, 512], f32, tag="av", bufs=2, name="av0")
                    av1 = psp.tile([P, 512], f32, tag="av", bufs=2, name="av1")
                    for mt in range(MT):
                        mcols = slice(mt * P, (mt + 1) * P)
                        s = psp.tile([P, 1024], f32, tag="s", bufs=2, name="s")
                        nc.tensor.matmul(
                            s[:, 0:512],
                            kT_sb[0:DH, pair, mcols],
                            qT_sb[0:DH, pair, cols],
                            start=True, stop=True,
                        )
                        nc.tensor.matmul(
                            s[:, 512:1024],
                            kT_sb[DH:2 * DH, pair, mcols],
                            qT_sb[DH:2 * DH, pair, cols],
                            start=True, stop=True,
                        )
                        p = wp.tile([P, 1024], bf, tag="p", bufs=8, name="p")
                        nc.scalar.activation(p[:], s[:], Exp, scale=0.125)
                        if pair == 0 and lc == 0:
                            proj_v(mt)
                        nc.tensor.matmul(
                            av0[0:65, :],
                            vaug_sb[:, mt, h0 * 65:h0 * 65 + 65],
                            p[:, 0:512],
                            start=(mt == 0), stop=(mt == MT - 1),
                        )
                        nc.tensor.matmul(
                            av1[0:65, :],
                            vaug_sb[:, mt, h1 * 65:h1 * 65 + 65],
                            p[:, 512:1024],
                            start=(mt == 0), stop=(mt == MT - 1),
                        )
                        for fn in fill.get(mt, ()):
                            fn()
                    # stash unnormalized A^T; scatter denominator rows for
                    # a batched reciprocal (per-pair; the last pair uses a
                    # per-lc [2,512] tile so it can normalize immediately,
                    # and its final chunk takes the shortest path: direct
                    # reciprocal + multiply straight from PSUM).
                    tail = last and lc == LC4 - 1
                    if last and not tail:
                        dnl = wp.tile([2, 512], f32, tag="dnl", bufs=2, name="dnl")
                    for hidx, (rb, av) in enumerate(((0, av0), (DH, av1))):
                        if tail:
                            rt = wp.tile([1, 512], f32, tag="rt", bufs=2, name="rt")
                            nc.vector.reciprocal(rt[:], av[64:65, :])
                            rbb = wp.tile([DH, 512], f32, tag="rbbt", bufs=2, name="rbbt")
                            nc.gpsimd.partition_broadcast(rbb[:], rt[:])
                            nc.vector.tensor_mul(
                                out=at_sb[rb:rb + DH, pair, cols],
                                in0=av[0:DH, :],
                                in1=rbb[:],
                            )
                            continue
                        nc.vector.tensor_copy(
                            at_sb[rb:rb + DH, pair, cols], av[0:DH, :]
                        )
                        stg = wp.tile([1, 512], f32, tag="stg", bufs=4, name="stg")
                        nc.vector.tensor_copy(stg[:], av[64:65, :])
                        if last:
                            nc.sync.dma_start(dnl[hidx:hidx + 1, :], stg[:])
                        else:
                            nc.sync.dma_start(dn[2 * lc + hidx:2 * lc + hidx + 1, :], stg[:])
                    if last and not tail:
                        rdl = wp.tile([2, 512], f32, tag="rdl", bufs=2, name="rdl")
                        nc.vector.reciprocal(rdl[:], dnl[:])
                        for hidx, rb in ((0, 0), (1, DH)):
                            normalize(pair, lc, rdl, hidx, rb)
                if not last:
                    rd = wp.tile([8, 512], f32, tag="rd", bufs=2, name="rd")
                    nc.vector.reciprocal(rd[:], dn[:])
                    for lc in range(LC4):
                        for hidx, rb in ((0, 0), (1, DH)):
                            normalize(pair, lc, rd, 2 * lc + hidx, rb)
            for lt in range(12, 16):
                proj_o(lt)

    nc.compile()
    return nc


def get_nc():
    if "nc" not in _cache:
        _cache["nc"] = _build()
    return _cache["nc"]


def _pretile(a, p=P):
    """[T*p, F] -> [p, T*F] with row i holding concat over tiles t of a[t*p+i]."""
    t = a.shape[0] // p
    return np.ascontiguousarray(
        a.reshape(t, p, a.shape[1]).transpose(1, 0, 2).reshape(p, t * a.shape[1])
    )


def make_in_maps(x, W_q, W_k, W_v, W_o):
    import ml_dtypes

    bf = ml_dtypes.bfloat16
    x = np.asarray(x, dtype=np.float32)
    W_q = np.asarray(W_q, dtype=np.float32)
    W_k = np.asarray(W_k, dtype=np.float32)
    W_v = np.asarray(W_v, dtype=np.float32)
    W_o = np.asarray(W_o, dtype=np.float32)

    in_maps = []
    for core in range(8):
        b, hg = divmod(core, 2)
        rows = slice(hg * O, (hg + 1) * O)
        in_maps.append({
            "xT": _pretile(np.ascontiguousarray(x[b].T)).astype(bf),
            "wqT": _pretile(np.ascontiguousarray(W_q[rows].T)).astype(bf),
            "wkT": _pretile(np.ascontiguousarray(W_k[rows].T)).astype(bf),
            "wvT": _pretile(np.ascontiguousarray(W_v[rows].T)).astype(bf),
            "woT": _pretile(np.ascontiguousarray(W_o[:, rows].T)).astype(bf),
        })
    return in_maps


def kernel(x, W_q, W_k, W_v, W_o, b_o):
    from concourse.bass_utils import run_bass_kernel_spmd

    nc = get_nc()
    in_maps = make_in_maps(x, W_q, W_k, W_v, W_o)
    trace = bool(int(os.environ.get("ATTN_TRACE", "0")))
    res = run_bass_kernel_spmd(nc, in_maps, core_ids=list(range(8)), trace=trace)
    if trace and res.exec_time_ns is not None:
        _cache["exec_time_ns"] = res.exec_time_ns
        _cache["mean_exec_time_ns"] = res.mean_exec_time_ns

    b_o = np.asarray(b_o, dtype=np.float32)
    out = np.empty((B, L, D), np.float32)
    for b in range(B):
        # out dram is [128, 16, 1024]: row p, tile lt -> token lt*128+p
        acc = (res.results[2 * b]["out"] + res.results[2 * b + 1]["out"])
        out[b] = acc.reshape(P, MT, D).transpose(1, 0, 2).reshape(L, D) + b_o
    return out


# revision 32
# speedup vs baseline: 1.0293x; 1.0170x over previous
"""Multi-head attention (b=4, l=2048, d=1024, 16 heads) on 8 TRN2 NeuronCores.

Sharding: data parallel over the 4 batches x tensor parallel over 2 head
groups (8 heads each). core = 2*batch + head_group. Each core computes its
batch's attention for its 8 heads plus the partial W_o projection
(row-parallel); the host sums the two partials per batch and adds b_o.
No on-chip collectives needed.

Per-core layout (everything transposed so features sit on partitions):
  xT  [1024, 2048]; Q^T,K^T [512, 2048]; V in [m, o] layout.
  Per head pair (sharing an o-partition tile, rows 0-63 / 64-127):
    S^T = K Q^T for both heads into one [128, 1024] PSUM tile
    -> one exp on ScalarE -> P^T bf16
    -> A^T accumulated via V_aug (64 V cols + ones col -> denom in row 64)
  softmax denominators are DMA-scattered onto separate partitions and
  reciprocal'd in one batched DVE op; broadcast back via gpsimd.
  Y_partial = A^T.T-contract @ WoT -> [2048, 1024] fp32.

All DRAM operands are host-pretiled so every DMA is contiguous per
partition (32KB runs instead of 4KB), and emission interleaves the next
pair's QK projection / the O projection into the ACT-bound attention
stream so the TensorEngine never idles the ScalarEngine at phase edges.
"""

import os

import numpy as np

B = 4
L = 2048
D = 1024
P = 128
NH = 8          # heads per core
DH = 64
O = NH * DH     # 512 qkv dims per core
DT = D // P     # 8 d-tiles
OT = O // P     # 4 o-tiles (= head pairs)
MT = L // P     # 16 m-tiles
LC4 = L // 512  # 4 l-chunks of 512

_cache = {}


def _build():
    import concourse.tile as tile
    from concourse import bacc, mybir

    nc = bacc.Bacc("TRN2", target_bir_lowering=False, debug=False)
    bf = mybir.dt.bfloat16
    f32 = mybir.dt.float32
    Exp = mybir.ActivationFunctionType.Exp

    # host-pretiled: row p holds the concatenation over tiles (see
    # make_in_maps) so per-partition DRAM runs are large and contiguous.
    xT = nc.dram_tensor("xT", [P, DT * L], bf, kind="ExternalInput").ap()
    wqT = nc.dram_tensor("wqT", [P, DT * O], bf, kind="ExternalInput").ap()
    wkT = nc.dram_tensor("wkT", [P, DT * O], bf, kind="ExternalInput").ap()
    wvT = nc.dram_tensor("wvT", [P, DT * O], bf, kind="ExternalInput").ap()
    woT = nc.dram_tensor("woT", [P, OT * D], bf, kind="ExternalInput").ap()
    out = nc.dram_tensor("out", [P, MT * D], f32, kind="ExternalOutput").ap()
    xT3 = xT.rearrange("p (dt l) -> p dt l", l=L)
    out3 = out.rearrange("p (lt j) -> p lt j", j=D)

    with tile.TileContext(nc) as tc:
        with (
            tc.tile_pool(name="persist", bufs=1) as pp,
            tc.tile_pool(name="work", bufs=3) as wp,
            tc.tile_pool(name="psum", bufs=1, space="PSUM") as psp,
        ):
            xT_sb = pp.tile([P, DT, L], bf, name="xT_sb")
            wq_sb = pp.tile([P, DT, O], bf, name="wq_sb")
            wk_sb = pp.tile([P, DT, O], bf, name="wk_sb")
            wv_sb = pp.tile([P, DT, O], bf, name="wv_sb")
            wo_sb = pp.tile([P, OT, D], bf, name="wo_sb")
            qT_sb = pp.tile([P, OT, L], bf, name="qT_sb")
            kT_sb = pp.tile([P, OT, L], bf, name="kT_sb")
            vaug_sb = pp.tile([P, MT, NH * 65], bf, name="vaug_sb")
            at_sb = pp.tile([P, OT, L], bf, name="at_sb")

            nc.sync.dma_start(wq_sb[:], wqT[:].rearrange("p (dt o) -> p dt o", o=O))
            nc.sync.dma_start(wk_sb[:], wkT[:].rearrange("p (dt o) -> p dt o", o=O))
            for i in range(4):
                nc.sync.dma_start(xT_sb[:, 2 * i:2 * i + 2, :], xT3[:, 2 * i:2 * i + 2, :])
            nc.sync.dma_start(wv_sb[:], wvT[:].rearrange("p (dt o) -> p dt o", o=O))
            nc.sync.dma_start(wo_sb[:], woT[:].rearrange("p (ot j) -> p ot j", j=D))

            # ones columns for the softmax-denominator trick; V copies below
            # overwrite the first 64 columns of each head's 65-column block.
            nc.gpsimd.memset(vaug_sb[:], 1.0)

            def proj_qk_group(w_sb, dst, ot, lc):
                ps = psp.tile([P, 512], f32, tag="mm512", bufs=2, name="ps_qk")
                for dt in range(DT):
                    nc.tensor.matmul(
                        ps[:],
                        w_sb[:, dt, ot * P:(ot + 1) * P],
                        xT_sb[:, dt, lc * 512:(lc + 1) * 512],
                        start=(dt == 0),
                        stop=(dt == DT - 1),
                    )
                nc.vector.tensor_copy(dst[:, ot, lc * 512:(lc + 1) * 512], ps[:])

            def proj_v(mt):
                ps = psp.tile([P, 512], f32, tag="mm512", bufs=2, name="ps_v")
                for dt in range(DT):
                    nc.tensor.matmul(
                        ps[:],
                        xT_sb[:, dt, mt * P:(mt + 1) * P],
                        wv_sb[:, dt, :],
                        start=(dt == 0),
                        stop=(dt == DT - 1),
                    )
                for h in range(NH):
                    nc.vector.tensor_copy(
                        vaug_sb[:, mt, h * 65:h * 65 + 64],
                        ps[:, h * DH:(h + 1) * DH],
                    )

            def proj_o(lt):
                ob = wp.tile([P, 1024], f32, tag="ob", bufs=3, name="ob")
                for jc in range(2):
                    ps = psp.tile([P, 512], f32, tag="mm512", bufs=2, name="ps_o")
                    for ot in range(OT):
                        nc.tensor.matmul(
                            ps[:],
                            at_sb[:, ot, lt * P:(lt + 1) * P],
                            wo_sb[:, ot, jc * 512:(jc + 1) * 512],
                            start=(ot == 0),
                            stop=(ot == OT - 1),
                        )
                    nc.vector.tensor_copy(ob[:, jc * 512:(jc + 1) * 512], ps[:])
                nc.sync.dma_start(out3[:, lt, :], ob[:])

            # Only the two groups the very first S^T matmul needs are
            # emitted upfront; everything else streams in as filler work
            # inside the attention mt loops.
            proj_qk_group(wq_sb, qT_sb, 0, 0)
            proj_qk_group(wk_sb, kT_sb, 0, 0)

            def normalize(pair, lc, rd, r, rb):
                """at_sb[rb:rb+64, pair, lc-chunk] *= broadcast(rd[r])."""
                cols = slice(lc * 512, (lc + 1) * 512)
                rstg = wp.tile([1, 512], f32, tag="rstg", bufs=4, name="rstg")
                nc.sync.dma_start(rstg[:], rd[r:r + 1, :])
                rbb = wp.tile([P, 512], f32, tag="rbb", bufs=4, name="rbb")
                nc.gpsimd.partition_broadcast(rbb[:], rstg[:])
                nc.vector.tensor_mul(
                    out=at_sb[rb:rb + DH, pair, cols],
                    in0=at_sb[rb:rb + DH, pair, cols],
                    in1=rbb[rb:rb + DH, :],
                )

            # ---- attention, one head pair at a time -----------------------
            # Filler emissions (projection groups / O-projection tiles) are
            # woven INSIDE the mt loops so the PE always has matmul work
            # during the ACT-bound exp stream, never in bursts that stall
            # the ScalarEngine at chunk boundaries.
            def fillers_for(pair, lc):
                f = {}
                if pair == 0:
                    # pair 0's own remaining groups: kT m-groups just in
                    # time (S at m-tile mt reads kT group mt//4), qT for
                    # the next l-chunk midway through the previous one.
                    if lc == 0:
                        for g in (1, 2, 3):
                            f.setdefault(4 * g - 1, []).append(
                                lambda g=g: proj_qk_group(wk_sb, kT_sb, 0, g))
                    if lc < 3:
                        f.setdefault(6, []).append(
                            lambda lc=lc: proj_qk_group(wq_sb, qT_sb, 0, lc + 1))
                if pair < OT - 1:
                    # next pair's 8 groups: two per l-chunk, k-groups first
                    nxt = pair + 1
                    order = [(wq_sb, qT_sb, 0), (wk_sb, kT_sb, 0),
                             (wk_sb, kT_sb, 1), (wk_sb, kT_sb, 2),
                             (wk_sb, kT_sb, 3), (wq_sb, qT_sb, 1),
                             (wq_sb, qT_sb, 2), (wq_sb, qT_sb, 3)]
                    for i in (2 * lc, 2 * lc + 1):
                        w_sb, dst, g = order[i]
                        f.setdefault(5 + 6 * (i % 2), []).append(
                            lambda w_sb=w_sb, dst=dst, g=g:
                                proj_qk_group(w_sb, dst, nxt, g))
                if pair == OT - 1 and lc > 1:
                    # O-projection tiles unlocked two l-chunks back; keeping
                    # one chunk's worth in reserve leaves PE work for the
                    # final normalize chain after the last mt loop.
                    for i, lt in enumerate(range(4 * (lc - 2), 4 * (lc - 1))):
                        f.setdefault(4 * i + 3, []).append(
                            lambda lt=lt: proj_o(lt))
                return f

            for pair in range(OT):
                h0, h1 = 2 * pair, 2 * pair + 1
                last = pair == OT - 1
                if not last:
                    dn = wp.tile([8, 512], f32, tag="dn", bufs=2, name="dn")
                for lc in range(LC4):
                    cols = slice(lc * 512, (lc + 1) * 512)
                    fill = fillers_for(pair, lc)
                    av0 = psp.tile([P, 512], f32, tag="av", bufs=2, name="av0")
                    av1 = psp.tile([P, 512], f32, tag="av", bufs=2, name="av1")
                    for mt in range(MT):
                        mcols = slice(mt * P, (mt + 1) * P)
                        s = psp.tile([P, 1024], f32, tag="s", bufs=2, name="s")
                        nc.tensor.matmul(
                            s[:, 0:512],
                            kT_sb[0:DH, pair, mcols],
                            qT_sb[0:DH, pair, cols],
                            start=True, stop=True,
                        )
                        nc.tensor.matmul(
                            s[:, 512:1024],
                            kT_sb[DH:2 * DH, pair, mcols],
                            qT_sb[DH:2 * DH, pair, cols],
                            start=True, stop=True,
                        )
                        p = wp.tile([P, 1024], bf, tag="p", bufs=8, name="p")
                        nc.scalar.activation(p[:], s[:], Exp, scale=0.125)
                        if pair == 0 and lc == 0:
                            proj_v(mt)
                        nc.tensor.matmul(
                            av0[0:65, :],
                            vaug_sb[:, mt, h0 * 65:h0 * 65 + 65],
                            p[:, 0:512],
                            start=(mt == 0), stop=(mt == MT - 1),
                        )
                        nc.tensor.matmul(
                            av1[0:65, :],
                            vaug_sb[:, mt, h1 * 65:h1 * 65 + 65],
                            p[:, 512:1024],
                            start=(mt == 0), stop=(mt == MT - 1),
                        )
                        for fn in fill.get(mt, ()):
                            fn()
                    # stash unnormalized A^T; scatter denominator rows for
                    # a batched reciprocal (per-pair; the last pair uses a
                    # per-lc [2,512] tile so it can normalize immediately,
                    # and its final chunk takes the shortest path: direct
                    # reciprocal + multiply straight from PSUM).
                    tail = last and lc == LC4 - 1
                    if last and not tail:
                        dnl = wp.tile([2, 512], f32, tag="dnl", bufs=2, name="dnl")
                    for hidx, (rb, av) in enumerate(((0, av0), (DH, av1))):
                        if tail:
                            rt = wp.tile([1, 512], f32, tag="rt", bufs=2, name="rt")
                            nc.vector.reciprocal(rt[:], av[64:65, :])
                            rbb = wp.tile([DH, 512], f32, tag="rbbt", bufs=2, name="rbbt")
                            nc.gpsimd.partition_broadcast(rbb[:], rt[:])
                            nc.vector.tensor_mul(
                                out=at_sb[rb:rb + DH, pair, cols],
                                in0=av[0:DH, :],
                                in1=rbb[:],
                            )
                            continue
                        nc.vector.tensor_copy(
                            at_sb[rb:rb + DH, pair, cols], av[0:DH, :]
                        )
                        stg = wp.tile([1, 512], f32, tag="stg", bufs=4, name="stg")
                        nc.vector.tensor_copy(stg[:], av[64:65, :])
                        if last:
                            nc.sync.dma_start(dnl[hidx:hidx + 1, :], stg[:])
                        else:
                            nc.sync.dma_start(dn[2 * lc + hidx:2 * lc + hidx + 1, :], stg[:])
                    if last and not tail:
                        rdl = wp.tile([2, 512], f32, tag="rdl", bufs=2, name="rdl")
                        nc.vector.reciprocal(rdl[:], dnl[:])
                        for hidx, rb in ((0, 0), (1, DH)):
                            normalize(pair, lc, rdl, hidx, rb)
                if not last:
                    rd = wp.tile([8, 512], f32, tag="rd", bufs=2, name="rd")
                    nc.vector.reciprocal(rd[:], dn[:])
                    for lc in range(LC4):
                        for hidx, rb in ((0, 0), (1, DH)):
                            normalize(pair, lc, rd, 2 * lc + hidx, rb)
            for lt in range(8, 16):
                proj_o(lt)

    nc.compile()
    return nc


def get_nc():
    if "nc" not in _cache:
        _cache["nc"] = _build()
    return _cache["nc"]


def _pretile(a, p=P):
    """[T*p, F] -> [p, T*F] with row i holding concat over tiles t of a[t*p+i]."""
    t = a.shape[0] // p
    return np.ascontiguousarray(
        a.reshape(t, p, a.shape[1]).transpose(1, 0, 2).reshape(p, t * a.shape[1])
    )


def make_in_maps(x, W_q, W_k, W_v, W_o):
    import ml_dtypes

    bf = ml_dtypes.bfloat16
    x = np.asarray(x, dtype=np.float32)
    W_q = np.asarray(W_q, dtype=np.float32)
    W_k = np.asarray(W_k, dtype=np.float32)
    W_v = np.asarray(W_v, dtype=np.float32)
    W_o = np.asarray(W_o, dtype=np.float32)

    in_maps = []
    for core in range(8):
        b, hg = divmod(core, 2)
        rows = slice(hg * O, (hg + 1) * O)
        in_maps.append({
            "xT": _pretile(np.ascontiguousarray(x[b].T)).astype(bf),
            "wqT": _pretile(np.ascontiguousarray(W_q[rows].T)).astype(bf),
            "wkT": _pretile(np.ascontiguousarray(W_k[rows].T)).astype(bf),
            "wvT": _pretile(np.ascontiguousarray(W_v[rows].T)).astype(bf),
            "woT": _pretile(np.ascontiguousarray(W_o[:, rows].T)).astype(bf),
        })
    return in_maps


def kernel(x, W_q, W_k, W_v, W_o, b_o):
    from concourse.bass_utils import run_bass_kernel_spmd

    nc = get_nc()
    in_maps = make_in_maps(x, W_q, W_k, W_v, W_o)
    trace = bool(int(os.environ.get("ATTN_TRACE", "0")))
    res = run_bass_kernel_spmd(nc, in_maps, core_ids=list(range(8)), trace=trace)
    if trace and res.exec_time_ns is not None:
        _cache["exec_time_ns"] = res.exec_time_ns
        _cache["mean_exec_time_ns"] = res.mean_exec_time_ns

    b_o = np.asarray(b_o, dtype=np.float32)
    out = np.empty((B, L, D), np.float32)
    for b in range(B):
        # out dram is [128, 16, 1024]: row p, tile lt -> token lt*128+p
        acc = (res.results[2 * b]["out"] + res.results[2 * b + 1]["out"])
        out[b] = acc.reshape(P, MT, D).transpose(1, 0, 2).reshape(L, D) + b_o
    return out


# revision 34
# speedup vs baseline: 1.0297x; 1.0004x over previous
"""Multi-head attention (b=4, l=2048, d=1024, 16 heads) on 8 TRN2 NeuronCores.

Sharding: data parallel over the 4 batches x tensor parallel over 2 head
groups (8 heads each). core = 2*batch + head_group. Each core computes its
batch's attention for its 8 heads plus the partial W_o projection
(row-parallel); the host sums the two partials per batch and adds b_o.
No on-chip collectives needed.

Per-core layout (everything transposed so features sit on partitions):
  xT  [1024, 2048]; Q^T,K^T [512, 2048]; V in [m, o] layout.
  Per head pair (sharing an o-partition tile, rows 0-63 / 64-127):
    S^T = K Q^T for both heads into one [128, 1024] PSUM tile
    -> one exp on ScalarE -> P^T bf16
    -> A^T accumulated via V_aug (64 V cols + ones col -> denom in row 64)
  softmax denominators are DMA-scattered onto separate partitions and
  reciprocal'd in one batched DVE op; broadcast back via gpsimd.
  Y_partial = A^T.T-contract @ WoT -> [2048, 1024] fp32.

All DRAM operands are host-pretiled so every DMA is contiguous per
partition (32KB runs instead of 4KB), and emission interleaves the next
pair's QK projection / the O projection into the ACT-bound attention
stream so the TensorEngine never idles the ScalarEngine at phase edges.
"""

import os

import numpy as np

B = 4
L = 2048
D = 1024
P = 128
NH = 8          # heads per core
DH = 64
O = NH * DH     # 512 qkv dims per core
DT = D // P     # 8 d-tiles
OT = O // P     # 4 o-tiles (= head pairs)
MT = L // P     # 16 m-tiles
LC4 = L // 512  # 4 l-chunks of 512

_cache = {}


def _build():
    import concourse.tile as tile
    from concourse import bacc, mybir

    nc = bacc.Bacc("TRN2", target_bir_lowering=False, debug=False)
    bf = mybir.dt.bfloat16
    f32 = mybir.dt.float32
    Exp = mybir.ActivationFunctionType.Exp

    # host-pretiled: row p holds the concatenation over tiles (see
    # make_in_maps) so per-partition DRAM runs are large and contiguous.
    xT = nc.dram_tensor("xT", [P, DT * L], bf, kind="ExternalInput").ap()
    wqT = nc.dram_tensor("wqT", [P, DT * O], bf, kind="ExternalInput").ap()
    wkT = nc.dram_tensor("wkT", [P, DT * O], bf, kind="ExternalInput").ap()
    wvT = nc.dram_tensor("wvT", [P, DT * O], bf, kind="ExternalInput").ap()
    woT = nc.dram_tensor("woT", [P, OT * D], bf, kind="ExternalInput").ap()
    out = nc.dram_tensor("out", [P, MT * D], f32, kind="ExternalOutput").ap()
    xT3 = xT.rearrange("p (dt l) -> p dt l", l=L)
    out3 = out.rearrange("p (lt j) -> p lt j", j=D)

    with tile.TileContext(nc) as tc:
        with (
            tc.tile_pool(name="persist", bufs=1) as pp,
            tc.tile_pool(name="work", bufs=3) as wp,
            tc.tile_pool(name="psum", bufs=1, space="PSUM") as psp,
        ):
            xT_sb = pp.tile([P, DT, L], bf, name="xT_sb")
            wq_sb = pp.tile([P, DT, O], bf, name="wq_sb")
            wk_sb = pp.tile([P, DT, O], bf, name="wk_sb")
            wv_sb = pp.tile([P, DT, O], bf, name="wv_sb")
            wo_sb = pp.tile([P, OT, D], bf, name="wo_sb")
            qT_sb = pp.tile([P, OT, L], bf, name="qT_sb")
            kT_sb = pp.tile([P, OT, L], bf, name="kT_sb")
            vaug_sb = pp.tile([P, MT, NH * 65], bf, name="vaug_sb")
            at_sb = pp.tile([P, OT, L], bf, name="at_sb")

            nc.sync.dma_start(wq_sb[:], wqT[:].rearrange("p (dt o) -> p dt o", o=O))
            nc.sync.dma_start(wk_sb[:], wkT[:].rearrange("p (dt o) -> p dt o", o=O))
            for i in range(4):
                nc.sync.dma_start(xT_sb[:, 2 * i:2 * i + 2, :], xT3[:, 2 * i:2 * i + 2, :])
            nc.sync.dma_start(wv_sb[:], wvT[:].rearrange("p (dt o) -> p dt o", o=O))
            nc.sync.dma_start(wo_sb[:], woT[:].rearrange("p (ot j) -> p ot j", j=D))

            # ones columns for the softmax-denominator trick; V copies below
            # overwrite the first 64 columns of each head's 65-column block.
            nc.gpsimd.memset(vaug_sb[:], 1.0)

            def proj_qk_group(w_sb, dst, ot, lc):
                ps = psp.tile([P, 512], f32, tag="mm512", bufs=2, name="ps_qk")
                for dt in range(DT):
                    nc.tensor.matmul(
                        ps[:],
                        w_sb[:, dt, ot * P:(ot + 1) * P],
                        xT_sb[:, dt, lc * 512:(lc + 1) * 512],
                        start=(dt == 0),
                        stop=(dt == DT - 1),
                    )
                nc.vector.tensor_copy(dst[:, ot, lc * 512:(lc + 1) * 512], ps[:])

            def proj_v(mt):
                ps = psp.tile([P, 512], f32, tag="mm512", bufs=2, name="ps_v")
                for dt in range(DT):
                    nc.tensor.matmul(
                        ps[:],
                        xT_sb[:, dt, mt * P:(mt + 1) * P],
                        wv_sb[:, dt, :],
                        start=(dt == 0),
                        stop=(dt == DT - 1),
                    )
                for h in range(NH):
                    nc.vector.tensor_copy(
                        vaug_sb[:, mt, h * 65:h * 65 + 64],
                        ps[:, h * DH:(h + 1) * DH],
                    )

            def proj_o(lt):
                ob = wp.tile([P, 1024], f32, tag="ob", bufs=3, name="ob")
                for jc in range(2):
                    ps = psp.tile([P, 512], f32, tag="mm512", bufs=2, name="ps_o")
                    for ot in range(OT):
                        nc.tensor.matmul(
                            ps[:],
                            at_sb[:, ot, lt * P:(lt + 1) * P],
                            wo_sb[:, ot, jc * 512:(jc + 1) * 512],
                            start=(ot == 0),
                            stop=(ot == OT - 1),
                        )
                    nc.vector.tensor_copy(ob[:, jc * 512:(jc + 1) * 512], ps[:])
                nc.sync.dma_start(out3[:, lt, :], ob[:])

            # Only the two groups the very first S^T matmul needs are
            # emitted upfront; everything else streams in as filler work
            # inside the attention mt loops.
            proj_qk_group(wq_sb, qT_sb, 0, 0)
            proj_qk_group(wk_sb, kT_sb, 0, 0)

            def normalize(pair, lc, rd, r, rb):
                """at_sb[rb:rb+64, pair, lc-chunk] *= broadcast(rd[r])."""
                cols = slice(lc * 512, (lc + 1) * 512)
                rstg = wp.tile([1, 512], f32, tag="rstg", bufs=4, name="rstg")
                nc.sync.dma_start(rstg[:], rd[r:r + 1, :])
                rbb = wp.tile([P, 512], f32, tag="rbb", bufs=4, name="rbb")
                nc.gpsimd.partition_broadcast(rbb[:], rstg[:])
                nc.vector.tensor_mul(
                    out=at_sb[rb:rb + DH, pair, cols],
                    in0=at_sb[rb:rb + DH, pair, cols],
                    in1=rbb[rb:rb + DH, :],
                )

            # ---- attention, one head pair at a time -----------------------
            # Filler emissions (projection groups / O-projection tiles) are
            # woven INSIDE the mt loops so the PE always has matmul work
            # during the ACT-bound exp stream, never in bursts that stall
            # the ScalarEngine at chunk boundaries.
            def fillers_for(pair, lc):
                f = {}
                if pair == 0:
                    # pair 0's own remaining groups: kT m-groups just in
                    # time (S at m-tile mt reads kT group mt//4), qT for
                    # the next l-chunk midway through the previous one.
                    if lc == 0:
                        for g in (1, 2, 3):
                            f.setdefault(4 * g - 1, []).append(
                                lambda g=g: proj_qk_group(wk_sb, kT_sb, 0, g))
                    if lc < 3:
                        f.setdefault(6, []).append(
                            lambda lc=lc: proj_qk_group(wq_sb, qT_sb, 0, lc + 1))
                if pair < OT - 1:
                    # next pair's 8 groups: two per l-chunk, k-groups first
                    nxt = pair + 1
                    order = [(wq_sb, qT_sb, 0), (wk_sb, kT_sb, 0),
                             (wk_sb, kT_sb, 1), (wk_sb, kT_sb, 2),
                             (wk_sb, kT_sb, 3), (wq_sb, qT_sb, 1),
                             (wq_sb, qT_sb, 2), (wq_sb, qT_sb, 3)]
                    for i in (2 * lc, 2 * lc + 1):
                        w_sb, dst, g = order[i]
                        f.setdefault(5 + 6 * (i % 2), []).append(
                            lambda w_sb=w_sb, dst=dst, g=g:
                                proj_qk_group(w_sb, dst, nxt, g))
                if pair == OT - 1 and lc > 1:
                    # O-projection tiles unlocked two l-chunks back; keeping
                    # one chunk's worth in reserve leaves PE work for the
                    # final normalize chain after the last mt loop.
                    for i, lt in enumerate(range(4 * (lc - 2), 4 * (lc - 1))):
                        f.setdefault(4 * i + 3, []).append(
                            lambda lt=lt: proj_o(lt))
                return f

            for pair in range(OT):
                h0, h1 = 2 * pair, 2 * pair + 1
                last = pair == OT - 1
                if not last:
                    dn = wp.tile([8, 512], f32, tag="dn", bufs=2, name="dn")
                for lc in range(LC4):
                    cols = slice(lc * 512, (lc + 1) * 512)
                    fill = fillers_for(pair, lc)
                    av0 = psp.tile([P, 512], f32, tag="av", bufs=2, name="av0")
                    av1 = psp.tile([P, 512], f32, tag="av", bufs=2, name="av1")
                    for mt in range(MT):
                        mcols = slice(mt * P, (mt + 1) * P)
                        s = psp.tile([P, 1024], f32, tag="s", bufs=2, name="s")
                        nc.tensor.matmul(
                            s[:, 0:512],
                            kT_sb[0:DH, pair, mcols],
                            qT_sb[0:DH, pair, cols],
                            start=True, stop=True,
                        )
                        nc.tensor.matmul(
                            s[:, 512:1024],
                            kT_sb[DH:2 * DH, pair, mcols],
                            qT_sb[DH:2 * DH, pair, cols],
                            start=True, stop=True,
                        )
                        p = wp.tile([P, 1024], bf, tag="p", bufs=8, name="p")
                        nc.scalar.activation(p[:], s[:], Exp, scale=0.125)
                        if pair == 0 and lc == 0:
                            proj_v(mt)
                        nc.tensor.matmul(
                            av0[0:65, :],
                            vaug_sb[:, mt, h0 * 65:h0 * 65 + 65],
                            p[:, 0:512],
                            start=(mt == 0), stop=(mt == MT - 1),
                        )
                        nc.tensor.matmul(
                            av1[0:65, :],
                            vaug_sb[:, mt, h1 * 65:h1 * 65 + 65],
                            p[:, 512:1024],
                            start=(mt == 0), stop=(mt == MT - 1),
                        )
                        for fn in fill.get(mt, ()):
                            fn()
                    # stash unnormalized A^T; scatter denominator rows for
                    # a batched reciprocal (per-pair; the last pair uses a
                    # per-lc [2,512] tile so it can normalize immediately,
                    # and its final chunk takes the shortest path: direct
                    # reciprocal + multiply straight from PSUM).
                    tail = last and lc == LC4 - 1
                    if last and not tail:
                        dnl = wp.tile([2, 512], f32, tag="dnl", bufs=2, name="dnl")
                    for hidx, (rb, av) in enumerate(((0, av0), (DH, av1))):
                        if tail:
                            rt = wp.tile([1, 512], f32, tag="rt", bufs=2, name="rt")
                            nc.vector.reciprocal(rt[:], av[64:65, :])
                            rbb = wp.tile([DH, 512], f32, tag="rbbt", bufs=2, name="rbbt")
                            nc.gpsimd.partition_broadcast(rbb[:], rt[:])
                            nc.vector.tensor_mul(
                                out=at_sb[rb:rb + DH, pair, cols],
                                in0=av[0:DH, :],
                                in1=rbb[:],
                            )
                            continue
                        nc.vector.tensor_copy(
                            at_sb[rb:rb + DH, pair, cols], av[0:DH, :]
                        )
                        stg = wp.tile([1, 512], f32, tag="stg", bufs=4, name="stg")
                        nc.vector.tensor_copy(stg[:], av[64:65, :])
                        if last:
                            nc.sync.dma_start(dnl[hidx:hidx + 1, :], stg[:])
                        else:
                            nc.sync.dma_start(dn[2 * lc + hidx:2 * lc + hidx + 1, :], stg[:])
                    if last and not tail:
                        rdl = wp.tile([2, 512], f32, tag="rdl", bufs=2, name="rdl")
                        nc.vector.reciprocal(rdl[:], dnl[:])
                        for hidx, rb in ((0, 0), (1, DH)):
                            normalize(pair, lc, rdl, hidx, rb)
                if not last:
                    rd = wp.tile([8, 512], f32, tag="rd", bufs=2, name="rd")
                    nc.vector.reciprocal(rd[:], dn[:])
                    for lc in range(LC4):
                        for hidx, rb in ((0, 0), (1, DH)):
                            normalize(pair, lc, rd, 2 * lc + hidx, rb)
            for lt in range(8, 16):
                proj_o(lt)

    nc.compile()
    return nc


def get_nc():
    if "nc" not in _cache:
        _cache["nc"] = _build()
    return _cache["nc"]


def _pretile(a, p=P):
    """[T*p, F] -> [p, T*F] with row i holding concat over tiles t of a[t*p+i]."""
    t = a.shape[0] // p
    return np.ascontiguousarray(
        a.reshape(t, p, a.shape[1]).transpose(1, 0, 2).reshape(p, t * a.shape[1])
    )


def make_in_maps(x, W_q, W_k, W_v, W_o):
    import ml_dtypes

    bf = ml_dtypes.bfloat16
    x = np.asarray(x, dtype=np.float32)
    W_q = np.asarray(W_q, dtype=np.float32)
    W_k = np.asarray(W_k, dtype=np.float32)
    W_v = np.asarray(W_v, dtype=np.float32)
    W_o = np.asarray(W_o, dtype=np.float32)

    in_maps = []
    for core in range(8):
        b, hg = divmod(core, 2)
        rows = slice(hg * O, (hg + 1) * O)
        in_maps.append({
            "xT": _pretile(np.ascontiguousarray(x[b].T)).astype(bf),
            "wqT": _pretile(np.ascontiguousarray(W_q[rows].T)).astype(bf),
            "wkT": _pretile(np.ascontiguousarray(W_k[rows].T)).astype(bf),
            "wvT": _pretile(np.ascontiguousarray(W_v[rows].T)).astype(bf),
            "woT": _pretile(np.ascontiguousarray(W_o[:, rows].T)).astype(bf),
        })
    return in_maps


def kernel(x, W_q, W_k, W_v, W_o, b_o):
    from concourse.bass_utils import run_bass_kernel_spmd

    nc = get_nc()
    in_maps = make_in_maps(x, W_q, W_k, W_v, W_o)
    trace = bool(int(os.environ.get("ATTN_TRACE", "0")))
    res = run_bass_kernel_spmd(nc, in_maps, core_ids=list(range(8)), trace=trace)
    if trace and res.exec_time_ns is not None:
        _cache["exec_time_ns"] = res.exec_time_ns
        _cache["mean_exec_time_ns"] = res.mean_exec_time_ns

    b_o = np.asarray(b_o, dtype=np.float32)
    out = np.empty((B, L, D), np.float32)
    for b in range(B):
        # out dram is [128, 16, 1024]: row p, tile lt -> token lt*128+p
        acc = (res.results[2 * b]["out"] + res.results[2 * b + 1]["out"])
        out[b] = acc.reshape(P, MT, D).transpose(1, 0, 2).reshape(L, D) + b_o
    return out
